# revision 32
# baseline (speedup 1.0000x reference)
"""Multi-head attention kernel for Trainium2, sharded over 8 NeuronCores.

Problem: x[2,2048,1024] -> MHA(16 heads, dh=64) -> out[2,2048,512].

Sharding: core c handles batch b=c//4 and head-group g=c%4 (4 heads each).
Each core computes QKV for its heads, attention, and a partial output
projection through its 256-row slice of Wo. Host sums the 4 head-group
partials per batch and adds bo.

Per-core kernel design (all matmuls in float32r = FP22 multiply, fp32
accumulate — 1 cycle/row on the PE, ~1e-4 rel err; fp32r operands must be
produced pre-rounded, so f32r inputs are rounded on the host and on-chip
producers write f32r-dtype tiles):
  - x^T [din, s] arrives pre-transposed from the host (contraction for
    QKV is din), streamed by q-chunk so projections start on first bytes.
  - Q^T, K^T packed in one [128, q/k, pair, s] tile: head h at partition
    base 64*(h%2); scores^T tiles [k,q] come from lhsT=K^T slice,
    rhs=Q^T slice at the same base (distinct PE row-groups per head).
  - V stored natural [s, (head, dh+ones)]: each head has 64 V columns plus
    a ones column, so the attention matmul (lhsT=V_aug, rhs=exp(S^T))
    yields attn^T [64,q] rows 0-63 AND the softmax denominator in row 64.
  - softmax: exp on ScalarE with scale=1/8 folded in; no max subtraction
    (scores are bounded ~|2| for these inputs); normalization multiplies
    attn^T by a reciprocal row broadcast across partitions via a K=1
    ones-matmul.
  - out partial [s, 512] = attnT.T @ Wo_slice via lhsT=attnT tiles.
  - Emission order pipelines ScalarE's exp stream (the co-bottleneck with
    PE) against PE's projection matmuls: K/Q for heads 0-1 and V first,
    then heads 0-1 attention interleaves with K/Q for heads 2-3, and the
    output projection interleaves per q-chunk at the tail.
"""

import sys

sys.path.insert(0, "/opt/trn_rl_repo")

import numpy as np
from contextlib import ExitStack

# Problem shapes (hardcoded per the harness contract).
B = 2
S = 2048
DIN = 1024
H = 16
DH = 64
DMODEL = H * DH  # 1024
DOUT = 512
NCORES = 8

# Per-core shard shapes.
HPC = 4  # heads per core
DQ = HPC * DH  # 256: per-core QKV width
KT = DIN // 128  # 8  k-tiles over d_in
MT = DQ // 128  # 2  m-tiles over per-core dq
ST = S // 128  # 16 s-tiles
QC = S // 512  # 4  q-chunks of 512
KC = S // 128  # 16 k-tiles over sequence
VW = DH + 1  # 65: V columns per head incl. ones column


def build_program(repeat=1):
    from concourse import bacc, tile
    import concourse.bass as bass
    import concourse.mybir as mybir

    f32 = mybir.dt.float32
    f32r = mybir.dt.float32r
    Exp = mybir.ActivationFunctionType.Exp

    nc = bacc.Bacc("TRN2", target_bir_lowering=False, debug=False)

    x_d = nc.dram_tensor("x", [128, KT, S], f32r, kind="ExternalInput")
    wq_d = nc.dram_tensor("wq", [128, KT, DQ], f32r, kind="ExternalInput")
    wk_d = nc.dram_tensor("wk", [128, KT, DQ], f32r, kind="ExternalInput")
    wv_d = nc.dram_tensor("wv", [128, KT, DQ], f32r, kind="ExternalInput")
    bq_d = nc.dram_tensor("bq", [DH, HPC], f32, kind="ExternalInput")
    bk_d = nc.dram_tensor("bk", [DH, HPC], f32, kind="ExternalInput")
    bv_d = nc.dram_tensor("bv", [1, DQ], f32r, kind="ExternalInput")
    wo_d = nc.dram_tensor("wo", [128, MT, DOUT], f32r, kind="ExternalInput")
    out_d = nc.dram_tensor("out", [S, DOUT], f32, kind="ExternalOutput")

    with tile.TileContext(nc) as tc, ExitStack() as octx:
        consts = octx.enter_context(tc.tile_pool(name="consts", bufs=1))
        ones_f32 = consts.tile([128, 128], f32)
        nc.vector.memset(ones_f32[:], 1.0)
        ones = consts.tile([1, 128], f32r)
        nc.vector.tensor_copy(ones[:], ones_f32[0:1, :])
        ones16 = consts.tile([128, 16], f32r)
        nc.vector.tensor_copy(ones16[:], ones_f32[:, :16])
        bq_sb = consts.tile([DH, HPC], f32)
        bk_sb = consts.tile([DH, HPC], f32)
        bv_sb = consts.tile([1, DQ], f32r)
        nc.sync.dma_start(bq_sb[:], bq_d[:])
        nc.sync.dma_start(bk_sb[:], bk_d[:])
        nc.sync.dma_start(bv_sb[:], bv_d[:])
        wo_sb = consts.tile([128, MT, DOUT], f32r)
        nc.sync.dma_start(wo_sb[:], wo_d[:])

        # Persistent intermediates. Q^T and K^T share one full-partition
        # tile: head h lives at partition base 64*(h%2), pair index h//2.
        # An S^T matmul then has lhsT (K^T) and rhs (Q^T) at the SAME base
        # partition, which bass requires (and maps to PE row-groups).
        keep = octx.enter_context(tc.tile_pool(name="keep", bufs=1))
        qk_sb = keep.tile([128, 2, MT, S], f32r)  # [part, q/k, pair, s]
        v_sb = keep.tile([128, ST, HPC * VW], f32r)  # V natural + ones cols
        at_sb = keep.tile([128, MT, S], f32r)  # attn^T (dq on partitions)
        for h in range(HPC):  # ones column per head for the softmax sums
            nc.vector.tensor_copy(v_sb[:, :, h * VW + DH], ones16[:])

        for _rep in range(repeat):
            with ExitStack() as p12:
                xt_pool = p12.enter_context(tc.tile_pool(name="xt", bufs=1))
                xt_sb = xt_pool.tile([128, KT, S], f32r)  # x^T

                wts = p12.enter_context(tc.tile_pool(name="wts", bufs=1))
                wq_sb = wts.tile([128, KT, DQ], f32r)
                wk_sb = wts.tile([128, KT, DQ], f32r)
                wv_sb = wts.tile([128, KT, DQ], f32r)

                proj_ps = p12.enter_context(
                    tc.tile_pool(name="proj_ps", bufs=2, space="PSUM")
                )

                # ---- Lead-in: stream x^T by q-chunk; project K/Q (m=0)
                # and V per chunk, and start pair-0 qc-0 attention eighths
                # as soon as their K/Q/V regions land. x^T arrives from the
                # host pre-transposed, so there is no on-chip transpose.
                exps = p12.enter_context(tc.tile_pool(name="exps", bufs=3))
                small = p12.enter_context(tc.tile_pool(name="small", bufs=4))
                s_ps = p12.enter_context(
                    tc.tile_pool(name="s_ps", bufs=2, space="PSUM")
                )
                a_ps = p12.enter_context(
                    tc.tile_pool(name="a_ps", bufs=2, space="PSUM")
                )
                o_sb = p12.enter_context(tc.tile_pool(name="o_sb", bufs=3))

                def qk_proj(w_sb, b_sb, qki, m, qc):
                    """One q-chunk of the Q^T (qki=0) / K^T (qki=1) m-tile."""
                    ps = proj_ps.tile([128, 512], f32, tag="proj")
                    for k in range(KT):
                        nc.tensor.matmul(
                            ps[:],
                            w_sb[:, k, m * 128 : (m + 1) * 128],
                            xt_sb[:, k, qc * 512 : (qc + 1) * 512],
                            start=(k == 0),
                            stop=(k == KT - 1),
                        )
                    for j in range(2):
                        h = 2 * m + j
                        nc.vector.tensor_scalar_add(
                            qk_sb[
                                j * 64 : j * 64 + 64,
                                qki,
                                m,
                                qc * 512 : (qc + 1) * 512,
                            ],
                            ps[j * 64 : j * 64 + 64, :],
                            b_sb[:, h : h + 1],
                        )

                def v_proj_st(st):
                    """V rows for s-tile st (bias-seeded, per-head columns)."""
                    ps = proj_ps.tile([128, 512], f32, tag="proj")
                    nc.tensor.matmul(
                        ps[:, :DQ], ones[:, :128], bv_sb[:], start=True, stop=False
                    )
                    for k in range(KT):
                        nc.tensor.matmul(
                            ps[:, :DQ],
                            xt_sb[:, k, st * 128 : (st + 1) * 128],
                            wv_sb[:, k, :],
                            start=False,
                            stop=(k == KT - 1),
                        )
                    vdst = v_sb[:, st, :].rearrange("p (h c) -> p h c", h=HPC)[
                        :, :, :DH
                    ]
                    nc.vector.tensor_copy(
                        vdst, ps[:, :DQ].rearrange("p (h c) -> p h c", h=HPC)
                    )

                class AttnPair:
                    """Both heads of pair p (bases 0 and 64) for q-chunk qc.

                    Emitted in eighths of 2 sequence k-tiles: both heads' S
                    matmuls (adjacent, distinct PE row-groups via their base
                    partitions), a paired 2-bank exp per head on ScalarE,
                    then the eighth's attn matmuls."""

                    def __init__(self, p, qc):
                        self.p, self.qc = p, qc
                        self.qsl = slice(qc * 512, (qc + 1) * 512)
                        self.aps = [
                            a_ps.tile([VW, 512], f32, tag="a", name=f"ap{j}")
                            for j in range(2)
                        ]

                    def eighth(self, qq):
                        p = self.p
                        et = exps.tile([128, 2, 2, 512], f32r, tag="exps")
                        for j in range(2):
                            base = 64 * j
                            sp = s_ps.tile([128, 2, 512], f32, tag="s")
                            for i in range(2):
                                kt = 2 * qq + i
                                nc.tensor.matmul(
                                    sp[:, i, :],
                                    qk_sb[
                                        base : base + 64,
                                        1,
                                        p,
                                        kt * 128 : (kt + 1) * 128,
                                    ],
                                    qk_sb[base : base + 64, 0, p, self.qsl],
                                    start=True,
                                    stop=True,
                                )
                            nc.scalar.activation(
                                et[:, j, :, :],
                                sp[:],
                                Exp,
                                scale=1.0 / np.sqrt(DH),
                            )
                        for i in range(2):
                            kt = 2 * qq + i
                            for j in range(2):
                                h = 2 * p + j
                                nc.tensor.matmul(
                                    self.aps[j][:],
                                    v_sb[:, kt, h * VW : (h + 1) * VW],
                                    et[:, j, i, :],
                                    start=(kt == 0),
                                    stop=(kt == KC - 1),
                                )

                    def finish(self):
                        for j in range(2):
                            ap = self.aps[j]
                            rec = small.tile([1, 512], f32r, tag="rec")
                            with nc.allow_low_precision(
                                reason="fp22 recip is plenty"
                            ):
                                nc.vector.reciprocal(rec[:], ap[DH : DH + 1, :])
                            rb = proj_ps.tile([128, 512], f32, tag="proj")
                            nc.tensor.matmul(
                                rb[:DH, :],
                                ones[:, :DH],
                                rec[:],
                                start=True,
                                stop=True,
                            )
                            rb_sb = small.tile([DH, 512], f32, tag="rb_sb")
                            nc.vector.tensor_copy(rb_sb[:], rb[:DH, :])
                            nc.vector.tensor_tensor(
                                at_sb[64 * j : 64 * j + 64, self.p, self.qsl],
                                ap[:DH, :],
                                rb_sb[:],
                                bass.mybir.AluOpType.mult,
                            )

                def attention_pair(p, qc, fillers=None):
                    apair = AttnPair(p, qc)
                    for qq in range(8):
                        apair.eighth(qq)
                        if fillers and qq % 2 == 1 and fillers[qq // 2]:
                            fillers[qq // 2]()
                    apair.finish()

                def out_proj_m(m):
                    """Output partial for s-tile m."""
                    ps = proj_ps.tile([128, DOUT], f32, tag="proj")
                    for k2 in range(MT):
                        nc.tensor.matmul(
                            ps[:],
                            at_sb[:, k2, m * 128 : (m + 1) * 128],
                            wo_sb[:, k2, :],
                            start=(k2 == 0),
                            stop=(k2 == MT - 1),
                        )
                    ot = o_sb.tile([128, DOUT], f32, tag="ot")
                    nc.vector.tensor_copy(ot[:], ps[:])
                    nc.sync.dma_start(out_d[m * 128 : (m + 1) * 128, :], ot[:])

                def KQ(w, b, qki, m, qc):
                    return lambda: qk_proj(w, b, qki, m, qc)

                # Chunked lead-in: per q-chunk of x^T, project K/Q (m=0) and
                # V, then run pair-0 qc-0 attention eighths for the k-tiles
                # that chunk covers.
                pair00 = AttnPair(0, 0)
                for qch in range(QC):
                    qsl = slice(qch * 512, (qch + 1) * 512)
                    if qch == 0:
                        # Split the first x^T chunk and pull only the m=0
                        # halves of Wk/Wq so the first projection matmuls
                        # start as early as the DMA stream allows.
                        nc.sync.dma_start(
                            xt_sb[:, :4, qsl], x_d[:, :4, qsl]
                        )
                        nc.sync.dma_start(wk_sb[:, :, :128], wk_d[:, :, :128])
                        nc.sync.dma_start(
                            xt_sb[:, 4:, qsl], x_d[:, 4:, qsl]
                        )
                        nc.sync.dma_start(wq_sb[:, :, :128], wq_d[:, :, :128])
                        nc.sync.dma_start(wv_sb[:], wv_d[:])
                    else:
                        nc.sync.dma_start(xt_sb[:, :, qsl], x_d[:, :, qsl])
                    if qch == 1:
                        nc.sync.dma_start(wk_sb[:, :, 128:], wk_d[:, :, 128:])
                    elif qch == 2:
                        nc.sync.dma_start(wq_sb[:, :, 128:], wq_d[:, :, 128:])
                    qk_proj(wk_sb, bk_sb, 1, 0, qch)
                    qk_proj(wq_sb, bq_sb, 0, 0, qch)
                    for st in range(4 * qch, 4 * qch + 4):
                        v_proj_st(st)
                    pair00.eighth(2 * qch)
                    pair00.eighth(2 * qch + 1)
                pair00.finish()

                attention_pair(
                    0,
                    1,
                    fillers=[
                        KQ(wk_sb, bk_sb, 1, 1, 0),
                        KQ(wk_sb, bk_sb, 1, 1, 1),
                        KQ(wk_sb, bk_sb, 1, 1, 2),
                        KQ(wk_sb, bk_sb, 1, 1, 3),
                    ],
                )
                attention_pair(
                    0,
                    2,
                    fillers=[
                        KQ(wq_sb, bq_sb, 0, 1, 0),
                        KQ(wq_sb, bq_sb, 0, 1, 1),
                        KQ(wq_sb, bq_sb, 0, 1, 2),
                        KQ(wq_sb, bq_sb, 0, 1, 3),
                    ],
                )
                attention_pair(0, 3)
                attention_pair(1, 0)
                for qc in range(1, QC):
                    attention_pair(
                        1,
                        qc,
                        fillers=[
                            (lambda m=m: out_proj_m(m))
                            for m in range(4 * (qc - 1), 4 * qc)
                        ],
                    )
                for m in range(12, 16):
                    out_proj_m(m)

    nc.compile()
    return nc


def round_fp22(a):
    """Round f32 to FP22 (e10m11-representable: 11 mantissa bits, RNE).

    The PE reads float32r operands by truncating to FP22; pre-rounding on
    the host makes the truncation an identity (and the BIR verifier demands
    fp32r matmul operands be produced pre-rounded)."""
    u = np.ascontiguousarray(a, dtype=np.float32).view(np.uint32)
    keep = u & np.uint32(0xFFFFF000)
    rnd = (u & np.uint32(0x00000FFF)) + ((u >> np.uint32(12)) & np.uint32(1))
    out = keep + np.where(rnd > np.uint32(0x800), np.uint32(0x1000), np.uint32(0))
    return out.view(np.float32)


def shard_inputs(inputs):
    """Build the 8 per-core input maps: core c -> batch c//4, head-group c%4."""
    x = np.asarray(inputs["x"], dtype=np.float32)
    Wq = np.asarray(inputs["Wq"], dtype=np.float32)
    Wk = np.asarray(inputs["Wk"], dtype=np.float32)
    Wv = np.asarray(inputs["Wv"], dtype=np.float32)
    bq = np.asarray(inputs["bq"], dtype=np.float32)
    bk = np.asarray(inputs["bk"], dtype=np.float32)
    bv = np.asarray(inputs["bv"], dtype=np.float32)
    Wo = np.asarray(inputs["Wo"], dtype=np.float32)

    def wslice(W, g):
        # [1024, 256] -> [128, KT, 256] (partition-major k-tiles)
        w = W[:, g * DQ : (g + 1) * DQ]
        return round_fp22(w.reshape(KT, 128, DQ).transpose(1, 0, 2))

    def bcol(b, g):
        # [256] -> [64, 4]: per-head per-partition columns
        return np.ascontiguousarray(b[g * DQ : (g + 1) * DQ].reshape(HPC, DH).T)

    in_maps = []
    for c in range(NCORES):
        b, g = divmod(c, HPC)
        wo = Wo[g * DQ : (g + 1) * DQ, :]
        in_maps.append(
            {
                "x": round_fp22(
                    x[b].T.reshape(KT, 128, S).transpose(1, 0, 2)
                ),
                "wq": wslice(Wq, g),
                "wk": wslice(Wk, g),
                "wv": wslice(Wv, g),
                "bq": bcol(bq, g),
                "bk": bcol(bk, g),
                "bv": round_fp22(bv[g * DQ : (g + 1) * DQ].reshape(1, DQ)),
                "wo": round_fp22(wo.reshape(MT, 128, DOUT).transpose(1, 0, 2)),
            }
        )
    return in_maps


_PROGRAM_CACHE = []


def run_on_hw(inputs, trace=False):
    from concourse.bass_utils import run_bass_kernel_spmd

    if not _PROGRAM_CACHE:
        _PROGRAM_CACHE.append(build_program(1))
    nc = _PROGRAM_CACHE[0]
    in_maps = shard_inputs(inputs)
    # trace=True needs the axon NTFF hook (antenv.axon_hooks), absent here.
    res = run_bass_kernel_spmd(nc, in_maps, list(range(NCORES)), trace=False)
    bo = np.asarray(inputs["bo"], dtype=np.float32)
    out = np.zeros((B, S, DOUT), dtype=np.float32)
    for c in range(NCORES):
        out[c // HPC] += res.results[c]["out"]
    out += bo
    return out, res


def kernel(**inputs):
    out, _ = run_on_hw(inputs, trace=False)
    return out


# revision 38
# speedup vs baseline: 23.9831x; 23.9831x over previous
"""Multi-head attention kernel for Trainium2, sharded over 8 NeuronCores.

Problem: x[2,2048,1024] -> MHA(16 heads, dh=64) -> out[2,2048,512].

Sharding: core c handles batch b=c//4 and head-group g=c%4 (4 heads each).
Each core computes QKV for its heads, attention, and a partial output
projection through its 256-row slice of Wo. Host sums the 4 head-group
partials per batch and adds bo.

Per-core kernel design (all matmuls in float32r = FP22 multiply, fp32
accumulate — 1 cycle/row on the PE, ~1e-4 rel err; fp32r operands must be
produced pre-rounded, so f32r inputs are rounded on the host and on-chip
producers write f32r-dtype tiles):
  - x^T [din, s] arrives pre-transposed from the host (contraction for
    QKV is din), streamed by q-chunk so projections start on first bytes.
  - Q^T, K^T packed in one [128, q/k, pair, s] tile: head h at partition
    base 64*(h%2); scores^T tiles [k,q] come from lhsT=K^T slice,
    rhs=Q^T slice at the same base (distinct PE row-groups per head).
  - V stored natural [s, (head, dh+ones)]: each head has 64 V columns plus
    a ones column, so the attention matmul (lhsT=V_aug, rhs=exp(S^T))
    yields attn^T [64,q] rows 0-63 AND the softmax denominator in row 64.
  - softmax: exp on ScalarE with scale=1/8 folded in; no max subtraction
    (scores are bounded ~|2| for these inputs); normalization multiplies
    attn^T by a reciprocal row broadcast across partitions via a K=1
    ones-matmul.
  - out partial [s, 512] = attnT.T @ Wo_slice via lhsT=attnT tiles.
  - Emission order pipelines ScalarE's exp stream (the co-bottleneck with
    PE) against PE's projection matmuls: K/Q for heads 0-1 and V first,
    then heads 0-1 attention interleaves with K/Q for heads 2-3, and the
    output projection interleaves per q-chunk at the tail.
"""

import sys

sys.path.insert(0, "/opt/trn_rl_repo")

import numpy as np
from contextlib import ExitStack

# Problem shapes (hardcoded per the harness contract).
B = 2
S = 2048
DIN = 1024
H = 16
DH = 64
DMODEL = H * DH  # 1024
DOUT = 512
NCORES = 8

# Per-core shard shapes.
HPC = 4  # heads per core
DQ = HPC * DH  # 256: per-core QKV width
KT = DIN // 128  # 8  k-tiles over d_in
MT = DQ // 128  # 2  m-tiles over per-core dq
ST = S // 128  # 16 s-tiles
QC = S // 512  # 4  q-chunks of 512
KC = S // 128  # 16 k-tiles over sequence
VW = DH + 1  # 65: V columns per head incl. ones column


def build_program(repeat=1):
    from concourse import bacc, tile
    import concourse.bass as bass
    import concourse.mybir as mybir

    f32 = mybir.dt.float32
    f32r = mybir.dt.float32r
    Exp = mybir.ActivationFunctionType.Exp

    nc = bacc.Bacc("TRN2", target_bir_lowering=False, debug=False)

    x_d = nc.dram_tensor("x", [QC, 128, KT, 512], f32r, kind="ExternalInput")
    wq_d = nc.dram_tensor("wq", [128, KT, DQ], f32r, kind="ExternalInput")
    wk_d = nc.dram_tensor("wk", [128, KT, DQ], f32r, kind="ExternalInput")
    wv_d = nc.dram_tensor("wv", [128, KT, DQ], f32r, kind="ExternalInput")
    bq_d = nc.dram_tensor("bq", [DH, HPC], f32, kind="ExternalInput")
    bk_d = nc.dram_tensor("bk", [DH, HPC], f32, kind="ExternalInput")
    bv_d = nc.dram_tensor("bv", [1, DQ], f32r, kind="ExternalInput")
    wo_d = nc.dram_tensor("wo", [128, MT, DOUT], f32r, kind="ExternalInput")
    out_d = nc.dram_tensor("out", [S, DOUT], f32, kind="ExternalOutput")

    with tile.TileContext(nc) as tc, ExitStack() as octx:
        consts = octx.enter_context(tc.tile_pool(name="consts", bufs=1))
        ones_f32 = consts.tile([128, 128], f32)
        nc.vector.memset(ones_f32[:], 1.0)
        ones = consts.tile([1, 128], f32r)
        nc.vector.tensor_copy(ones[:], ones_f32[0:1, :])
        ones16 = consts.tile([128, 16], f32r)
        nc.vector.tensor_copy(ones16[:], ones_f32[:, :16])
        bq_sb = consts.tile([DH, HPC], f32)
        bk_sb = consts.tile([DH, HPC], f32)
        bv_sb = consts.tile([1, DQ], f32r)
        nc.sync.dma_start(bq_sb[:], bq_d[:])
        nc.sync.dma_start(bk_sb[:], bk_d[:])
        nc.sync.dma_start(bv_sb[:], bv_d[:])
        wo_sb = consts.tile([128, MT, DOUT], f32r)
        nc.sync.dma_start(wo_sb[:], wo_d[:])

        # Persistent intermediates. Q^T and K^T share one full-partition
        # tile: head h lives at partition base 64*(h%2), pair index h//2.
        # An S^T matmul then has lhsT (K^T) and rhs (Q^T) at the SAME base
        # partition, which bass requires (and maps to PE row-groups).
        keep = octx.enter_context(tc.tile_pool(name="keep", bufs=1))
        qk_sb = keep.tile([128, 2, MT, S], f32r)  # [part, q/k, pair, s]
        v_sb = keep.tile([128, ST, HPC * VW], f32r)  # V natural + ones cols
        at_sb = keep.tile([128, MT, S], f32r)  # attn^T (dq on partitions)
        for h in range(HPC):  # ones column per head for the softmax sums
            nc.vector.tensor_copy(v_sb[:, :, h * VW + DH], ones16[:])

        for _rep in range(repeat):
            with ExitStack() as p12:
                xt_pool = p12.enter_context(tc.tile_pool(name="xt", bufs=1))
                xt_sb = xt_pool.tile([128, KT, S], f32r)  # x^T

                wts = p12.enter_context(tc.tile_pool(name="wts", bufs=1))
                wq_sb = wts.tile([128, KT, DQ], f32r)
                wk_sb = wts.tile([128, KT, DQ], f32r)
                wv_sb = wts.tile([128, KT, DQ], f32r)

                proj_ps = p12.enter_context(
                    tc.tile_pool(name="proj_ps", bufs=2, space="PSUM")
                )

                # ---- Lead-in: stream x^T by q-chunk; project K/Q (m=0)
                # and V per chunk, and start pair-0 qc-0 attention eighths
                # as soon as their K/Q/V regions land. x^T arrives from the
                # host pre-transposed, so there is no on-chip transpose.
                exps = p12.enter_context(tc.tile_pool(name="exps", bufs=3))
                small = p12.enter_context(tc.tile_pool(name="small", bufs=4))
                s_ps = p12.enter_context(
                    tc.tile_pool(name="s_ps", bufs=2, space="PSUM")
                )
                a_ps = p12.enter_context(
                    tc.tile_pool(name="a_ps", bufs=2, space="PSUM")
                )
                o_sb = p12.enter_context(tc.tile_pool(name="o_sb", bufs=3))

                def qk_proj(w_sb, b_sb, qki, m, qc):
                    """One q-chunk of the Q^T (qki=0) / K^T (qki=1) m-tile."""
                    ps = proj_ps.tile([128, 512], f32, tag="proj")
                    for k in range(KT):
                        nc.tensor.matmul(
                            ps[:],
                            w_sb[:, k, m * 128 : (m + 1) * 128],
                            xt_sb[:, k, qc * 512 : (qc + 1) * 512],
                            start=(k == 0),
                            stop=(k == KT - 1),
                        )
                    for j in range(2):
                        h = 2 * m + j
                        nc.vector.tensor_scalar_add(
                            qk_sb[
                                j * 64 : j * 64 + 64,
                                qki,
                                m,
                                qc * 512 : (qc + 1) * 512,
                            ],
                            ps[j * 64 : j * 64 + 64, :],
                            b_sb[:, h : h + 1],
                        )

                def v_proj_st(st):
                    """V rows for s-tile st (bias-seeded, per-head columns)."""
                    ps = proj_ps.tile([128, 512], f32, tag="proj")
                    nc.tensor.matmul(
                        ps[:, :DQ], ones[:, :128], bv_sb[:], start=True, stop=False
                    )
                    for k in range(KT):
                        nc.tensor.matmul(
                            ps[:, :DQ],
                            xt_sb[:, k, st * 128 : (st + 1) * 128],
                            wv_sb[:, k, :],
                            start=False,
                            stop=(k == KT - 1),
                        )
                    vdst = v_sb[:, st, :].rearrange("p (h c) -> p h c", h=HPC)[
                        :, :, :DH
                    ]
                    nc.vector.tensor_copy(
                        vdst, ps[:, :DQ].rearrange("p (h c) -> p h c", h=HPC)
                    )

                class AttnPair:
                    """Both heads of pair p (bases 0 and 64) for q-chunk qc.

                    Emitted in eighths of 2 sequence k-tiles: both heads' S
                    matmuls (adjacent, distinct PE row-groups via their base
                    partitions), a paired 2-bank exp per head on ScalarE,
                    then the eighth's attn matmuls."""

                    def __init__(self, p, qc):
                        self.p, self.qc = p, qc
                        self.ets = {}
                        self.qsl = slice(qc * 512, (qc + 1) * 512)
                        self.aps = [
                            a_ps.tile([VW, 512], f32, tag="a", name=f"ap{j}")
                            for j in range(2)
                        ]

                    def s_exp(self, qq):
                        p = self.p
                        et = exps.tile([128, 2, 2, 512], f32r, tag="exps")
                        self.ets[qq] = et
                        for j in range(2):
                            base = 64 * j
                            sp = s_ps.tile([128, 2, 512], f32, tag="s")
                            for i in range(2):
                                kt = 2 * qq + i
                                nc.tensor.matmul(
                                    sp[:, i, :],
                                    qk_sb[
                                        base : base + 64,
                                        1,
                                        p,
                                        kt * 128 : (kt + 1) * 128,
                                    ],
                                    qk_sb[base : base + 64, 0, p, self.qsl],
                                    start=True,
                                    stop=True,
                                )
                            nc.scalar.activation(
                                et[:, j, :, :],
                                sp[:],
                                Exp,
                                scale=1.0 / np.sqrt(DH),
                            )
                    def attn(self, qq):
                        et = self.ets.pop(qq)
                        for i in range(2):
                            kt = 2 * qq + i
                            for j in range(2):
                                h = 2 * self.p + j
                                nc.tensor.matmul(
                                    self.aps[j][:],
                                    v_sb[:, kt, h * VW : (h + 1) * VW],
                                    et[:, j, i, :],
                                    start=(kt == 0),
                                    stop=(kt == KC - 1),
                                )

                    def eighth(self, qq):
                        self.s_exp(qq)
                        self.attn(qq)

                    def finish(self):
                        for j in range(2):
                            ap = self.aps[j]
                            rec = small.tile([1, 512], f32r, tag="rec")
                            with nc.allow_low_precision(
                                reason="fp22 recip is plenty"
                            ):
                                nc.vector.reciprocal(rec[:], ap[DH : DH + 1, :])
                            rb = proj_ps.tile([128, 512], f32, tag="proj")
                            nc.tensor.matmul(
                                rb[:DH, :],
                                ones[:, :DH],
                                rec[:],
                                start=True,
                                stop=True,
                            )
                            rb_sb = small.tile([DH, 512], f32, tag="rb_sb")
                            nc.vector.tensor_copy(rb_sb[:], rb[:DH, :])
                            nc.vector.tensor_tensor(
                                at_sb[64 * j : 64 * j + 64, self.p, self.qsl],
                                ap[:DH, :],
                                rb_sb[:],
                                bass.mybir.AluOpType.mult,
                            )

                def attention_pair(p, qc, fillers=None):
                    apair = AttnPair(p, qc)
                    for qq in range(8):
                        apair.eighth(qq)
                        if fillers and qq % 2 == 1 and fillers[qq // 2]:
                            fillers[qq // 2]()
                    apair.finish()

                def out_proj_m(m):
                    """Output partial for s-tile m."""
                    ps = proj_ps.tile([128, DOUT], f32, tag="proj")
                    for k2 in range(MT):
                        nc.tensor.matmul(
                            ps[:],
                            at_sb[:, k2, m * 128 : (m + 1) * 128],
                            wo_sb[:, k2, :],
                            start=(k2 == 0),
                            stop=(k2 == MT - 1),
                        )
                    ot = o_sb.tile([128, DOUT], f32, tag="ot")
                    nc.vector.tensor_copy(ot[:], ps[:])
                    nc.sync.dma_start(out_d[m * 128 : (m + 1) * 128, :], ot[:])

                def KQ(w, b, qki, m, qc):
                    return lambda: qk_proj(w, b, qki, m, qc)

                # Chunked lead-in: per q-chunk of x^T, project K/Q (m=0) and
                # V, then run pair-0 qc-0 attention eighths for the k-tiles
                # that chunk covers.
                pair00 = AttnPair(0, 0)
                for qch in range(QC):
                    qsl = slice(qch * 512, (qch + 1) * 512)
                    if qch == 0:
                        # Split the first x^T chunk and pull only the m=0
                        # halves of Wk/Wq so the first projection matmuls
                        # start as early as the DMA stream allows.
                        nc.sync.dma_start(
                            xt_sb[:, :4, qsl], x_d[qch, :, :4, :]
                        )
                        nc.sync.dma_start(wk_sb[:, :, :128], wk_d[:, :, :128])
                        nc.sync.dma_start(
                            xt_sb[:, 4:, qsl], x_d[qch, :, 4:, :]
                        )
                        nc.sync.dma_start(wq_sb[:, :, :128], wq_d[:, :, :128])
                        nc.sync.dma_start(wv_sb[:], wv_d[:])
                    else:
                        nc.sync.dma_start(xt_sb[:, :, qsl], x_d[qch])
                    if qch == 1:
                        nc.sync.dma_start(wk_sb[:, :, 128:], wk_d[:, :, 128:])
                    elif qch == 2:
                        nc.sync.dma_start(wq_sb[:, :, 128:], wq_d[:, :, 128:])
                    qk_proj(wk_sb, bk_sb, 1, 0, qch)
                    if qch == 0:
                        qk_proj(wq_sb, bq_sb, 0, 0, 0)
                    pair00.s_exp(2 * qch)
                    pair00.s_exp(2 * qch + 1)
                    if qch > 0:
                        qk_proj(wq_sb, bq_sb, 0, 0, qch)
                    for st in range(4 * qch, 4 * qch + 4):
                        v_proj_st(st)
                    pair00.attn(2 * qch)
                    pair00.attn(2 * qch + 1)
                pair00.finish()

                attention_pair(
                    0,
                    1,
                    fillers=[
                        KQ(wk_sb, bk_sb, 1, 1, 0),
                        KQ(wk_sb, bk_sb, 1, 1, 1),
                        KQ(wk_sb, bk_sb, 1, 1, 2),
                        KQ(wk_sb, bk_sb, 1, 1, 3),
                    ],
                )
                attention_pair(
                    0,
                    2,
                    fillers=[
                        KQ(wq_sb, bq_sb, 0, 1, 0),
                        KQ(wq_sb, bq_sb, 0, 1, 1),
                        KQ(wq_sb, bq_sb, 0, 1, 2),
                        KQ(wq_sb, bq_sb, 0, 1, 3),
                    ],
                )
                attention_pair(0, 3)
                attention_pair(1, 0)
                for qc in range(1, QC):
                    attention_pair(
                        1,
                        qc,
                        fillers=[
                            (lambda m=m: out_proj_m(m))
                            for m in range(4 * (qc - 1), 4 * qc)
                        ],
                    )
                for m in range(12, 16):
                    out_proj_m(m)

    nc.compile()
    return nc


def round_fp22(a):
    """Round f32 to FP22 (e10m11-representable: 11 mantissa bits, RNE).

    The PE reads float32r operands by truncating to FP22; pre-rounding on
    the host makes the truncation an identity (and the BIR verifier demands
    fp32r matmul operands be produced pre-rounded)."""
    u = np.ascontiguousarray(a, dtype=np.float32).view(np.uint32)
    keep = u & np.uint32(0xFFFFF000)
    rnd = (u & np.uint32(0x00000FFF)) + ((u >> np.uint32(12)) & np.uint32(1))
    out = keep + np.where(rnd > np.uint32(0x800), np.uint32(0x1000), np.uint32(0))
    return out.view(np.float32)


def shard_inputs(inputs):
    """Build the 8 per-core input maps: core c -> batch c//4, head-group c%4."""
    x = np.asarray(inputs["x"], dtype=np.float32)
    Wq = np.asarray(inputs["Wq"], dtype=np.float32)
    Wk = np.asarray(inputs["Wk"], dtype=np.float32)
    Wv = np.asarray(inputs["Wv"], dtype=np.float32)
    bq = np.asarray(inputs["bq"], dtype=np.float32)
    bk = np.asarray(inputs["bk"], dtype=np.float32)
    bv = np.asarray(inputs["bv"], dtype=np.float32)
    Wo = np.asarray(inputs["Wo"], dtype=np.float32)

    def wslice(W, g):
        # [1024, 256] -> [128, KT, 256] (partition-major k-tiles)
        w = W[:, g * DQ : (g + 1) * DQ]
        return round_fp22(w.reshape(KT, 128, DQ).transpose(1, 0, 2))

    def bcol(b, g):
        # [256] -> [64, 4]: per-head per-partition columns
        return np.ascontiguousarray(b[g * DQ : (g + 1) * DQ].reshape(HPC, DH).T)

    in_maps = []
    for c in range(NCORES):
        b, g = divmod(c, HPC)
        wo = Wo[g * DQ : (g + 1) * DQ, :]
        in_maps.append(
            {
                "x": round_fp22(
                    x[b].T.reshape(KT, 128, QC, 512).transpose(2, 1, 0, 3)
                ),
                "wq": wslice(Wq, g),
                "wk": wslice(Wk, g),
                "wv": wslice(Wv, g),
                "bq": bcol(bq, g),
                "bk": bcol(bk, g),
                "bv": round_fp22(bv[g * DQ : (g + 1) * DQ].reshape(1, DQ)),
                "wo": round_fp22(wo.reshape(MT, 128, DOUT).transpose(1, 0, 2)),
            }
        )
    return in_maps


_PROGRAM_CACHE = []


def run_on_hw(inputs, trace=False):
    from concourse.bass_utils import run_bass_kernel_spmd

    if not _PROGRAM_CACHE:
        _PROGRAM_CACHE.append(build_program(1))
    nc = _PROGRAM_CACHE[0]
    in_maps = shard_inputs(inputs)
    # trace=True needs the axon NTFF hook (antenv.axon_hooks), absent here.
    res = run_bass_kernel_spmd(nc, in_maps, list(range(NCORES)), trace=False)
    bo = np.asarray(inputs["bo"], dtype=np.float32)
    out = np.zeros((B, S, DOUT), dtype=np.float32)
    for c in range(NCORES):
        out[c // HPC] += res.results[c]["out"]
    out += bo
    return out, res


def kernel(**inputs):
    out, _ = run_on_hw(inputs, trace=False)
    return out
